# revision 6
# baseline (speedup 1.0000x reference)
"""Trainium2 Bass kernel for nn_ModelSimplest (4D conv -> relu -> linear -> sigmoid).

fp8 DoubleRow, folded-boff + wavefront ramp + a-paired tails + tuned DMA order.

Per (a, oi): 4212 contraction rows r = (boff, k, l), J-shift baked into SBUF
tiles.  16 full 256-row DR matmuls (u<16) + (even a) one 116x2-row DR tail
matmul pairing (a, a+1) -> 215 matmuls per (t, oi), 384 cols each.

x tile per (t, ia): [128, 17, 2, 6, 64] fp8 — u<16 main units
(rows r = u*256 + g*128 + p), u=16 = tail block (p<116: g=0 rows
(boff=12, kl=208+p) of ia, g=1 same rows of ia+1; zero above).
tfm[a]: [128, 17, 2, 112] — u16 (even a) = tail-pair stationary.

Schedule: wavefront ramp over cells a+oi<5, then a-major main loop.  DMA
issue order keeps all ramp stationaries ahead of later x tiles.
"""
import sys
from contextlib import ExitStack

import numpy as np

sys.path.insert(0, "/opt/trn_rl_repo")

from concourse import bacc, bass, mybir, tile  # noqa: E402
from concourse.bass_utils import run_bass_kernel_spmd  # noqa: E402

KK = 13
S_IN = 18
S_OUT = 6
N_CORES = 8
B_TOTAL = 1024
B_CORE = B_TOTAL // N_CORES
B_SUB = 64
N_SUB = B_CORE // B_SUB
NCH = 3
NM = NCH * S_OUT * S_OUT              # 108
NMP = 112
NROW = KK * S_IN * S_IN               # 4212
NU = 16
NUX = 17                              # 16 main units + tail slot
NTAIL = NROW - NU * 256               # 116
WSCALE = 256.0
NSLOT = 10
RAMP_W = 5

F32 = mybir.dt.float32
BF16 = mybir.dt.bfloat16
FP8 = mybir.dt.float8e4
DR = mybir.MatmulPerfMode.DoubleRow

_CACHE = {}


def _build_nc():
    nc = bacc.Bacc(None, target_bir_lowering=False)

    xf = nc.dram_tensor("xf", [N_SUB, S_IN, 128, NUX, 2, S_OUT, B_SUB], FP8,
                        kind="ExternalInput")
    tfm = nc.dram_tensor("tfm", [KK, 128, NUX, 2, NMP], FP8,
                         kind="ExternalInput")
    wl = nc.dram_tensor("wl", [NM, S_OUT * S_OUT], BF16, kind="ExternalInput")
    bias4 = nc.dram_tensor("bias4", [NM, 1], F32, kind="ExternalInput")
    blin = nc.dram_tensor("blin", [1, 1], F32, kind="ExternalInput")
    out = nc.dram_tensor("out", [1, B_CORE], F32, kind="ExternalOutput")

    with tile.TileContext(nc) as tc, ExitStack() as ctx:
        cpool = ctx.enter_context(tc.tile_pool(name="consts", bufs=1))
        wl_sb = cpool.tile([NM, S_OUT * S_OUT], BF16)
        bias_sb = cpool.tile([NM, 1], F32)
        blin_sb = cpool.tile([1, 1], F32)
        consts_loaded = []

        def load_consts():
            if not consts_loaded:
                nc.sync.dma_start(bias_sb[:], bias4[:])
                nc.sync.dma_start(blin_sb[:], blin[:])
                consts_loaded.append(True)

        xpool = ctx.enter_context(tc.tile_pool(name="xs", bufs=1))
        twpool = ctx.enter_context(tc.tile_pool(name="tws", bufs=1))
        tw_tiles = {}

        def get_tw(a):
            if a not in tw_tiles:
                twt = twpool.tile([128, NUX, 2, NMP], FP8, tag=f"tfm{a}",
                                  name=f"tfm{a}")
                nc.sync.dma_start(twt[:], tfm[a])
                tw_tiles[a] = twt
            return tw_tiles[a]

        x_tiles = {}

        def load_x(t, ia, split=False):
            if (t, ia) in x_tiles:
                return
            xt = xpool.tile([128, NUX, 2, S_OUT, B_SUB], FP8,
                            tag=f"x{ia % NSLOT}", name=f"x_{t}_{ia}")
            if split:
                nc.sync.dma_start(xt[:, 0:1], xf[t, ia, :, 0:1])
                nc.sync.dma_start(xt[:, 1:6], xf[t, ia, :, 1:6])
                nc.sync.dma_start(xt[:, 6:NUX], xf[t, ia, :, 6:NUX])
            else:
                nc.sync.dma_start(xt[:], xf[t, ia])
            x_tiles[(t, ia)] = xt

        pspool = ctx.enter_context(
            tc.tile_pool(name="ps", bufs=1, space=bass.MemorySpace.PSUM))
        hpool = ctx.enter_context(tc.tile_pool(name="hs", bufs=1))
        opool = ctx.enter_context(tc.tile_pool(name="outs", bufs=2))

        pending = []

        def epilogue_oi(te, pse, i, lg):
            h = hpool.tile([NM, S_OUT, B_SUB], BF16, tag=f"h{i}",
                           name=f"h{i}_{te}")
            nc.scalar.activation(
                h[:], pse[i][:],
                mybir.ActivationFunctionType.Relu,
                bias=bias_sb[:],
            )
            for j in range(S_OUT):
                nc.tensor.matmul(
                    lg[:],
                    wl_sb[:, i * S_OUT + j:i * S_OUT + j + 1],
                    h[:, j, :],
                    start=(i == 0 and j == 0),
                    stop=(i == S_OUT - 1 and j == S_OUT - 1),
                )

        def epilogue_fin(te, lg):
            ot = opool.tile([1, B_SUB], F32, tag="ot", name=f"ot_{te}")
            nc.scalar.activation(
                ot[:], lg[:],
                mybir.ActivationFunctionType.Sigmoid,
                bias=blin_sb[:],
            )
            nc.sync.dma_start(out[:, te * B_SUB:(te + 1) * B_SUB], ot[:])

        def emit_epilogue():
            te, pse = pending.pop(0)
            lg = pspool.tile([1, B_SUB], F32, tag="lg", name=f"lg_{te}")
            for i in range(S_OUT):
                epilogue_oi(te, pse, i, lg)
            epilogue_fin(te, lg)

        def cell(t, a, oi, ps):
            xt = x_tiles[(t, a + oi)]
            twt = tw_tiles[a]
            for u in range(NU):
                nc.tensor.matmul(
                    ps[oi][:],
                    twt[:, u, :, 0:NM],
                    xt[:, u, :, :, :],
                    start=(a == 0 and u == 0),
                    stop=False,
                    perf_mode=DR,
                )
            if a % 2 == 0:
                nc.tensor.matmul(
                    ps[oi][:],
                    twt[0:NTAIL, NU, :, 0:NM],
                    xt[0:NTAIL, NU, :, :, :],
                    start=False,
                    stop=(a == KK - 1),
                    perf_mode=DR,
                )

        # HAM warmup: junk matmuls on wl_sb while the first x tile streams
        # in, so the PE clock gate is already at 8/8 when real work lands.
        nc.sync.dma_start(wl_sb[:], wl[:])
        ps_warm = pspool.tile([1, S_OUT * S_OUT], F32, tag="lg",
                              name="ps_warm")
        for _ in range(48):
            nc.tensor.matmul(ps_warm[:], wl_sb[:, 0:1], wl_sb[:, :],
                             start=True, stop=True)

        for t in range(N_SUB):
            ps = [
                pspool.tile([NM, S_OUT, B_SUB], F32, tag=f"ps{i}",
                            name=f"ps{i}_{t}")
                for i in range(S_OUT)
            ]
            # DMA issue order: critical path first, all ramp stationaries
            # ahead of the later x tiles.
            get_tw(0)
            load_x(t, 0, split=(t == 0))
            get_tw(1)
            for ia in range(1, S_OUT):
                load_x(t, ia)
            for a in range(2, S_OUT):
                get_tw(a)
            load_consts()
            for ia in range(S_OUT, NSLOT):
                load_x(t, ia)
            # wavefront ramp
            for w in range(RAMP_W):
                for a in range(w + 1):
                    cell(t, a, w - a, ps)
                if w == 4 and pending:
                    emit_epilogue()
            # main loop
            for a in range(KK):
                get_tw(a)
                if a + 1 < KK:
                    get_tw(a + 1)
                nxt = a - 1 + NSLOT
                if a >= 1:
                    if nxt < S_IN:
                        load_x(t, nxt)
                    elif t + 1 < N_SUB:
                        load_x(t + 1, nxt - S_IN)
                last_t = (t == N_SUB - 1)
                if last_t and a == KK - 1:
                    # interleave the final epilogue with the last cells:
                    # ps[oi] is complete right after cell (12, oi).
                    lg = pspool.tile([1, B_SUB], F32, tag="lg",
                                     name=f"lg_{t}")
                    for oi in range(S_OUT):
                        cell(t, a, oi, ps)
                        epilogue_oi(t, ps, oi, lg)
                    epilogue_fin(t, lg)
                else:
                    for oi in range(S_OUT):
                        if a + oi >= RAMP_W:
                            cell(t, a, oi, ps)
            if not (t == N_SUB - 1):
                pending.append((t, ps))

        while pending:
            emit_epilogue()

    nc.compile()
    return nc


try:
    import ml_dtypes
    np_bf16 = ml_dtypes.bfloat16
    np_fp8 = ml_dtypes.float8_e4m3
except ImportError:  # pragma: no cover
    raise


def _prep_inputs(x, W4, b4, Wlin, blin):
    B = x.shape[0]
    r_main = np.arange(NU * 256).reshape(NU, 2, 128)
    boff_m = r_main // 324
    kl_m = r_main % 324
    kl_t = 208 + np.arange(NTAIL)

    xt = np.ascontiguousarray(
        x[:, 0].transpose(3, 4, 1, 2, 0)).reshape(324, S_IN, S_IN, B)
    xt8 = xt.astype(np_fp8)

    # main units: [u, g, p, j, ia, B] -> [ia, p, u, g, j, B]
    jj = boff_m[..., None] + np.arange(S_OUT)
    xm_all = xt8[kl_m[..., None], :, jj, :]
    xm_all = np.ascontiguousarray(xm_all.transpose(4, 2, 0, 1, 3, 5))

    # tail block: [ia, p, g, j, B]
    base = xt8[kl_t, :, 12:12 + S_OUT, :].transpose(1, 0, 2, 3)
    xtl_all = np.zeros((S_IN, NTAIL, 2, S_OUT, B), np_fp8)
    xtl_all[:, :, 0] = base
    xtl_all[:S_IN - 1, :, 1] = base[1:]

    # combined xf [ia, p, u(17), g, j, B]
    xf_all = np.zeros((S_IN, 128, NUX, 2, S_OUT, B), np_fp8)
    xf_all[:, :, :NU] = xm_all
    xf_all[:, :NTAIL, NU] = xtl_all

    T_flat = np.zeros((324, KK, KK, NM), np.float32)
    kl = np.arange(324)
    k_in_v = kl // S_IN
    l_in_v = kl % S_IN
    W4t = W4[:, 0].transpose(0, 3, 4, 1, 2)
    for ch in range(NCH):
        for kp in range(S_OUT):
            for lp in range(S_OUT):
                m = ch * 36 + kp * 6 + lp
                dk = k_in_v - kp
                dl = l_in_v - lp
                valid = (dk >= 0) & (dk < KK) & (dl >= 0) & (dl < KK)
                T_flat[valid, :, :, m] = W4t[ch, dk[valid], dl[valid]]
    Tq = (T_flat * WSCALE).astype(np_fp8)

    tfm_np = np.zeros((KK, 128, NUX, 2, NMP), np_fp8)
    tgt = Tq[kl_m, :, boff_m, :]             # [u, g, p, a, m]
    tfm_np[:, :, :NU, :, :NM] = tgt.transpose(3, 2, 0, 1, 4)
    tailT = Tq[kl_t, :, 12, :]               # [p, a, m]
    for a in range(0, KK, 2):
        tfm_np[a, :NTAIL, NU, 0, :NM] = tailT[:, a]
        if a + 1 < KK:
            tfm_np[a, :NTAIL, NU, 1, :NM] = tailT[:, a + 1]

    m_idx = np.arange(NM)
    ch_idx = m_idx // 36
    rem = m_idx % 36
    i_idx = np.arange(S_OUT)
    j_idx = np.arange(S_OUT)
    feat = (ch_idx[:, None, None] * 1296 + i_idx[None, :, None] * 216
            + j_idx[None, None, :] * 36 + rem[:, None, None])
    wl_np = (Wlin[0, feat].reshape(NM, S_OUT * S_OUT)
             / WSCALE).astype(np_bf16)

    bias4_np = np.ascontiguousarray(
        (b4[m_idx // 36] * WSCALE).astype(np.float32).reshape(NM, 1))
    blin_np = np.asarray(blin, np.float32).reshape(1, 1)
    return xf_all, tfm_np, wl_np, bias4_np, blin_np


def kernel(x, W4, b4, Wlin, blin, _profile=False):
    x = np.asarray(x)
    W4 = np.asarray(W4)
    b4 = np.asarray(b4)
    Wlin = np.asarray(Wlin)
    blin = np.asarray(blin)

    xf_all, tfm_np, wl_np, bias4_np, blin_np = _prep_inputs(
        x, W4, b4, Wlin, blin)

    if "nc" not in _CACHE:
        _CACHE["nc"] = _build_nc()
    nc = _CACHE["nc"]

    in_maps = []
    for core in range(N_CORES):
        b0 = core * B_CORE
        xc = xf_all[..., b0:b0 + B_CORE].reshape(
            S_IN, 128, NUX, 2, S_OUT, N_SUB, B_SUB)
        xc = np.ascontiguousarray(xc.transpose(5, 0, 1, 2, 3, 4, 6))
        in_maps.append({
            "xf": xc,
            "tfm": tfm_np,
            "wl": wl_np,
            "bias4": bias4_np,
            "blin": blin_np,
        })

    res = run_bass_kernel_spmd(
        nc, in_maps, core_ids=list(range(N_CORES)), trace=_profile)
    outs = [res.results[i]["out"].reshape(B_CORE) for i in range(N_CORES)]
    full = np.concatenate(outs).reshape(B_TOTAL, 1).astype(np.float32)
    if _profile:
        return full, res
    return full
